# revision 25
# baseline (speedup 1.0000x reference)
"""Multi-head attention (16 heads, d=64, d_model=1024, SL=2048, BS=2) on 8
Trainium2 NeuronCores.

Sharding: core c handles batch b = c // 4 and heads [4*(c%4), 4*(c%4)+4).
Each core computes a partial output y_c[2048, 1024] (its 4 heads' contribution
through Wo for its batch); the host sums the 4 fp16 partials per batch.

v3 design:
- fp16 QK path / bf16 PV path matmuls (1 cycle/row vs fp32r's 2).
- Scores: K=64 contraction, 2-way PE row tiling (heads at partitions
  0-63 / 64-127 issue back-to-back, run concurrently).
- exp on ACT: [128,1024] PSUM->SBUF bf16, one instr per (pair, kt).
  ACT is the bottleneck engine (~143us busy); the whole schedule exists
  to start it early and keep it gap-free.
- AV: stationary V_h [128,64] col-tiled 2-way into au[0:64]/au[64:128];
  denominators via 4-way col-tiled [128,1]-ones matmuls.
- Slot pipeline: scores stream leads; AV/den stream trails by LAG slots;
  K-proj groups (256-col) chase piece-wise DMA arrivals; V-proj,
  Q-proj, normalize, o-proj spread into slots as extras.
- Host-side relayout so every DMA is a contiguous block; inputs split
  across the sync + scalar hardware DMA queues; Y written as fp16.
"""

import os
import sys
for _p in ("/opt/trn_rl_repo", "/root/.axon_site/_ro/trn_rl_repo"):
    if os.path.isdir(_p) and _p not in sys.path:
        sys.path.insert(0, _p)

import numpy as np

import concourse.bass as bass
import concourse.tile as tile
from concourse import bacc, mybir
from concourse.bass_utils import run_bass_kernel_spmd

N_CORES = 8
SL = 2048
BS = 2
DM = 1024          # d_model
H = 16             # total heads
DH = 64            # head dim
HPC = 4            # heads per core
IC = HPC * DH      # per-core inner dim = 256
F32 = mybir.dt.float32
BF16 = mybir.dt.bfloat16
F16 = mybir.dt.float16
Exp = mybir.ActivationFunctionType.Exp

N_DMC = DM // 128          # 8 d_model chunks
N_KT = SL // 128           # 16 k tiles
N_QC = SL // 512           # 4 q chunks
N_KG = 4                   # K-proj 512-col groups
LAG = 6                    # slots the AV/den stream trails the scores stream


def _enable_ldw_opt():
    """walrus ships with --enable-ldw-opt=false; background-buffer weight
    loads are the only way to hide LDWEIGHTS under full-array matmuls."""
    import concourse.bass_utils as bu
    if getattr(bu, "_ldw_patched", False):
        return
    orig = bu.run_command

    def patched(argv, **kw):
        argv = ["--enable-ldw-opt=true" if a == "--enable-ldw-opt=false" else a
                for a in argv]
        return orig(argv, **kw)

    bu.run_command = patched
    bu._ldw_patched = True


def build_kernel():
    if os.environ.get("LDW_OPT") == "1":
        _enable_ldw_opt()
    nc = bacc.Bacc("TRN2", target_bir_lowering=False, debug=False,
                   num_devices=N_CORES)
    # host pre-layouts (all contiguous DMAs):
    #  qT: [4 qc][1024 dm][512 q]    kT: [8 g][1024 dm][256 k]
    #  vT: [16 g][1024 dm][128 k]    w*: [128 p][8c * 256f]  wo: [256, 1024]
    qT = nc.dram_tensor("qT", [N_QC * DM, 512], F16, kind="ExternalInput").ap()
    kT = nc.dram_tensor("kT", [N_KG * DM, 512], F16, kind="ExternalInput").ap()
    vT = nc.dram_tensor("vT", [N_KT * DM, 128], BF16, kind="ExternalInput").ap()
    wqT = nc.dram_tensor("wqT", [128, N_DMC * IC], F16, kind="ExternalInput").ap()
    wkT = nc.dram_tensor("wkT", [128, N_DMC * IC], F16, kind="ExternalInput").ap()
    wvT = nc.dram_tensor("wvT", [128, N_DMC * IC], BF16, kind="ExternalInput").ap()
    woT = nc.dram_tensor("woT", [IC, DM], F16, kind="ExternalInput").ap()
    Y = nc.dram_tensor("Y", [SL, DM], F16, kind="ExternalOutput").ap()
    dbg = None
    if os.environ.get("KDBG") == "1":
        dbg = {nm: nc.dram_tensor(nm, shp, dt, kind="ExternalOutput").ap()
               for nm, shp, dt in (("DKT0", [128, SL], F16),
                                   ("DQT0", [128, SL], F16),
                                   ("DV", [128, N_KT * IC], BF16),
                                   ("DAT0", [128, SL], F16))}

    with tile.TileContext(nc) as tc:
        _build_body(nc, tc, qT, kT, vT, wqT, wkT, wvT, woT, Y, dbg)
    nc.compile()
    return nc


def _build_body(nc, tc, qT, kT, vT, wqT, wkT, wvT, woT, Y, dbg=None):
    import contextlib
    ctx = contextlib.ExitStack()
    with ctx:
        wpool = ctx.enter_context(tc.tile_pool(name="w", bufs=1))
        xin = ctx.enter_context(tc.tile_pool(name="xin", bufs=1))
        xqp = ctx.enter_context(tc.tile_pool(name="xqp", bufs=1))
        qk = ctx.enter_context(tc.tile_pool(name="qk", bufs=1))
        vpool = ctx.enter_context(tc.tile_pool(name="v", bufs=1))
        pp = ctx.enter_context(tc.tile_pool(name="pp", bufs=14))
        pps = ctx.enter_context(tc.tile_pool(name="pps", bufs=4))
        atp = ctx.enter_context(tc.tile_pool(name="at", bufs=1))
        ypool = ctx.enter_context(tc.tile_pool(name="y", bufs=2))
        misc = ctx.enter_context(tc.tile_pool(name="misc", bufs=2))
        # PSUM: scores 2x[128,1024] (4 banks) + au 2x[128,512] (2) +
        # den 1 + acc 1 = 8 banks exactly.
        ps_s = ctx.enter_context(tc.tile_pool(name="ps_s", bufs=2, space="PSUM"))
        ps_au = ctx.enter_context(tc.tile_pool(name="ps_au", bufs=2, space="PSUM"))
        ps_den = ctx.enter_context(tc.tile_pool(name="ps_den", bufs=1, space="PSUM"))
        ps_acc = ctx.enter_context(tc.tile_pool(name="ps_acc", bufs=1, space="PSUM"))

        # ---- weight tiles ----
        w_k = wpool.tile([128, N_DMC * IC], F16, tag="w_k", name="w_k")
        w_q = wpool.tile([128, N_DMC * IC], F16, tag="w_q", name="w_q")
        w_v = wpool.tile([128, N_DMC * IC], BF16, tag="w_v", name="w_v")
        wo_sb = [wpool.tile([128, DM], F16, tag=f"wo{i}", name=f"wo{i}")
                 for i in range(2)]

        ones = misc.tile([128, 1], BF16, tag="ones")
        nc.vector.memset(ones[:], 1.0)

        # ---- x group tiles: one rearranged contiguous DMA per group ----
        # kgrp[g]: [128, 8c*256] f16; qgrp[qc]: [128, 8c*512] f16;
        # vgrp[g]: [128, 8c*128] bf16.  All DMAs ride the sync hw queue in
        # need-order; the sync engine is otherwise idle so the ~0.7us
        # trigger cost per DMA never blocks a compute engine.
        kgrp = [xin.tile([128, N_DMC * 512], F16, tag=f"kg{g}", name=f"kg{g}")
                for g in range(N_KG)]
        qgrp = [xqp.tile([128, N_DMC * 512], F16, tag=f"qg{g}", name=f"qg{g}")
                for g in range(N_QC)]
        vgrp = [xin.tile([128, N_DMC * 128], BF16, tag=f"vg{g}", name=f"vg{g}")
                for g in range(N_KT)]

        def load_grp(dst, src_rows, w):
            nc.sync.dma_start(
                out=dst[:].rearrange("p (c f) -> p c f", c=N_DMC),
                in_=src_rows.rearrange("(c p) f -> p c f", p=128))

        def load_grp_half(dst, src_rows, h):
            nc.sync.dma_start(
                out=dst[:, h * 4 * 512:(h + 1) * 4 * 512].rearrange(
                    "p (c f) -> p c f", c=N_DMC // 2),
                in_=src_rows.rearrange("(c p) f -> p c f", p=128))

        nc.sync.dma_start(out=w_k[:], in_=wkT[:])
        load_grp_half(kgrp[0], kT[0:DM // 2, :], 0)
        load_grp_half(kgrp[0], kT[DM // 2:DM, :], 1)
        nc.sync.dma_start(out=w_q[:], in_=wqT[:])
        load_grp_half(qgrp[0], qT[0:DM // 2, :], 0)
        load_grp_half(qgrp[0], qT[DM // 2:DM, :], 1)
        def load_v(g):
            load_grp(vgrp[g], vT[g * DM:(g + 1) * DM, :], 128)

        load_grp(kgrp[1], kT[DM:2 * DM, :], 512)
        nc.sync.dma_start(out=w_v[:], in_=wvT[:])
        load_v(0); load_v(1)
        load_grp(kgrp[2], kT[2 * DM:3 * DM, :], 512)
        load_v(2); load_v(3)
        load_grp(kgrp[3], kT[3 * DM:4 * DM, :], 512)
        load_grp(qgrp[1], qT[DM:2 * DM, :], 512)
        for g in range(4, 12):
            load_v(g)
        load_grp(qgrp[2], qT[2 * DM:3 * DM, :], 512)
        for g in range(12, N_KT):
            load_v(g)
        load_grp(qgrp[3], qT[3 * DM:4 * DM, :], 512)
        for i in range(2):
            nc.sync.dma_start(out=wo_sb[i][:],
                              in_=woT[i * 128:(i + 1) * 128, :])

        # ---- long-lived activations ----
        QT = [qk.tile([128, SL], F16, tag=f"qt{p}", name=f"qt{p}") for p in range(2)]
        KT = [qk.tile([128, SL], F16, tag=f"kt{p}", name=f"kt{p}") for p in range(2)]
        AT = [atp.tile([128, SL], F16, tag=f"at{p}", name=f"at{p}") for p in range(2)]
        V = vpool.tile([128, N_KT * IC], BF16, tag="vsb")

        # ---- building blocks ----
        def kproj_g(pair, g, pool):
            acc = pool.tile([128, 512], F32, tag=pool._ktag, name="acc")
            for c in range(N_DMC):
                nc.tensor.matmul(
                    acc[:],
                    w_k[:, c * IC + pair * 128: c * IC + (pair + 1) * 128],
                    kgrp[g][:, c * 512:(c + 1) * 512],
                    start=(c == 0), stop=(c == N_DMC - 1))
                if c % 2 == 1:
                    yield
            nc.vector.tensor_copy(KT[pair][:, g * 512:(g + 1) * 512], acc[:])

        def qproj(pair, qc, pool):
            acc = pool.tile([128, 512], F32, tag=pool._ktag, name="acc")
            for c in range(N_DMC):
                nc.tensor.matmul(
                    acc[:],
                    w_q[:, c * IC + pair * 128: c * IC + (pair + 1) * 128],
                    qgrp[qc][:, c * 512:(c + 1) * 512],
                    start=(c == 0), stop=(c == N_DMC - 1))
                if c % 2 == 1:
                    yield
            nc.vector.tensor_copy(QT[pair][:, qc * 512:(qc + 1) * 512], acc[:])

        def vproj_g(g):
            acc = ps_acc.tile([128, 512], F32, tag="acc", name="acc")
            for c in range(N_DMC):
                nc.tensor.matmul(
                    acc[:, 0:IC],
                    vgrp[g][:, c * 128:(c + 1) * 128],
                    w_v[:, c * IC:(c + 1) * IC],
                    start=(c == 0), stop=(c == N_DMC - 1))
                if c % 2 == 1:
                    yield
            nc.vector.tensor_copy(V[:, g * IC:(g + 1) * IC], acc[:, 0:IC])

        def scores_wave(pair, qc, kt):
            s = ps_s.tile([128, 1024], F32, tag="sgrp", name="sgrp")
            for hl in range(2):
                nc.tensor.matmul(
                    s[:, hl * 512:(hl + 1) * 512],
                    KT[pair][hl * 64:(hl + 1) * 64, kt * 128:(kt + 1) * 128],
                    QT[pair][hl * 64:(hl + 1) * 64, qc * 512:(qc + 1) * 512],
                    start=True, stop=True)
            p = pp.tile([128, 1024], BF16, tag="p", name="p")
            nc.scalar.activation(p[:], s[:], Exp)
            return p

        def av_wave(au, pair, kt, p):
            for hl in range(2):
                nc.tensor.matmul(
                    au[hl * 64:(hl + 1) * 64, :],
                    V[:, kt * IC + pair * 128 + hl * 64:
                       kt * IC + pair * 128 + (hl + 1) * 64],
                    p[:, hl * 512:(hl + 1) * 512],
                    start=(kt == 0), stop=(kt == N_KT - 1))

        def den_wave(den, ktp, ps01, ps23):
            # ktp indexes kt pairs (0..7); ps* are DVE pair-sums of P
            for h in range(HPC):
                p = ps01 if h < 2 else ps23
                nc.tensor.matmul(
                    den[h * 32:h * 32 + 1, :],
                    ones[:],
                    p[:, (h % 2) * 512:(h % 2 + 1) * 512],
                    start=(ktp == 0), stop=(ktp == N_KT // 2 - 1),
                    tile_position=(0, h * 32))

        def normalize_pair(qc, pair, den, au):
            ls, rcs, rbs = [], [], []
            for hl in range(2):
                h = pair * 2 + hl
                l_sb = misc.tile([1, 512], F32, tag="l_sb", name="l_sb")
                nc.vector.tensor_copy(l_sb[:], den[h * 32:h * 32 + 1, :])
                ls.append(l_sb)
            for hl in range(2):
                rc = misc.tile([1, 512], F32, tag="rc", name="rc")
                nc.vector.reciprocal_approx_fast(out=rc[:], in_=ls[hl][:])
                rcs.append(rc)
            for hl in range(2):
                rb = misc.tile([64, 512], F32, tag="rb", name="rb")
                nc.gpsimd.partition_broadcast(rb[:], rcs[hl][:])
                rbs.append(rb)
            for hl in range(2):
                nc.vector.tensor_mul(
                    AT[pair][hl * 64:(hl + 1) * 64, qc * 512:(qc + 1) * 512],
                    au[hl * 64:(hl + 1) * 64, :], rbs[hl][:])

        ycur = {}

        def oproj_half(qt, mh, tail=False):
            if mh == 0:
                ycur[qt] = ypool.tile([128, DM], F16, tag="ysb", name="ysb")
            y_sb = ycur[qt]
            if tail:
                ypb = ps_s.tile([128, 1024], F32, tag="sgrp", name="sgrp")
                yp = ypb[:, 0:512]
            else:
                yp = ps_acc.tile([128, 512], F32, tag="acc", name="acc")
            for ich in range(2):
                nc.tensor.matmul(
                    yp[:],
                    AT[ich][:, qt * 128:(qt + 1) * 128],
                    wo_sb[ich][:, mh * 512:(mh + 1) * 512],
                    start=(ich == 0), stop=(ich == 1))
            nc.vector.tensor_copy(y_sb[:, mh * 512:(mh + 1) * 512], yp[:])
            if mh == 1:
                nc.sync.dma_start(out=Y[qt * 128:(qt + 1) * 128, :],
                                  in_=y_sb[:])
                del ycur[qt]

        # ================= slot pipeline =================
        # lead-in projections ping-pong through the (still free) au banks
        ps_au._ktag = "au"
        ps_acc._ktag = "acc"
        for gen in (kproj_g(0, 0, ps_au), qproj(0, 0, ps_au)):
            for _ in gen:
                pass
        # feed ACT as early as possible: first two pair01 scores waves go
        # out before pair23's projections even start
        pre01 = [scores_wave(0, 0, 0), scores_wave(0, 0, 1)]
        for gen in (kproj_g(1, 0, ps_au), qproj(1, 0, ps_au)):
            for _ in gen:
                pass

        P = {}                  # (qc, kt) -> (p01, p23)
        avst = {"au01": None, "au23": None, "den": None}

        def av_stream_elem(j):
            qc, kt = divmod(j, 16)
            if kt == 0:
                if qc > 0:
                    normalize_pair(qc - 1, 1, avst["den"], avst["au23"])
                avst["au01"] = ps_au.tile([128, 512], F32, tag="au", name="au01")
                avst["au23"] = ps_au.tile([128, 512], F32, tag="au", name="au23")
                avst["den"] = ps_den.tile([128, 512], F32, tag="den", name="den")
            p01, p23 = P[(qc, kt)]
            av_wave(avst["au01"], 0, kt, p01)
            av_wave(avst["au23"], 1, kt, p23)
            if kt % 2 == 1:
                q01, q23 = P.pop((qc, kt - 1))
                del P[(qc, kt)]
                ps01 = pps.tile([128, 1024], BF16, tag="psm", name="psm")
                nc.vector.tensor_add(ps01[:], q01[:], p01[:])
                ps23 = pps.tile([128, 1024], BF16, tag="psm", name="psm")
                nc.vector.tensor_add(ps23[:], q23[:], p23[:])
                den_wave(avst["den"], kt // 2, ps01, ps23)
            if kt == N_KT - 1:
                normalize_pair(qc, 0, avst["den"], avst["au01"])

        extras = {}

        def add_extra(s, fn):
            extras.setdefault(s, []).append(fn)

        # remaining K-proj groups chase DMA arrivals (needed by slot 4g)
        for g in range(1, N_KG):
            add_extra(4 * g - 4, (lambda g=g: (4 * g, kproj_g(0, g, ps_acc))))
            add_extra(4 * g - 2, (lambda g=g: (4 * g, kproj_g(1, g, ps_acc))))
        # V-proj groups: scheduled 2 slots ahead of their av consumer
        for g in range(N_KT):
            add_extra(g + LAG - 2, (lambda g=g: (g + LAG, vproj_g(g))))
        # Q-proj for qc 1..3, due before that qc's first scores slot
        for qc in range(1, N_QC):
            add_extra(16 * (qc - 1) + 9,
                      (lambda qc=qc: (16 * qc, qproj(0, qc, ps_acc))))
            add_extra(16 * (qc - 1) + 11,
                      (lambda qc=qc: (16 * qc, qproj(1, qc, ps_acc))))

        pending = []    # [due_slot, generator]

        def pump():
            if not pending:
                return
            try:
                next(pending[0][1])
            except StopIteration:
                pending.pop(0)

        def drain_due(s):
            i = 0
            while i < len(pending):
                due, gen = pending[i]
                if due <= s:
                    for _ in gen:
                        pass
                    pending.pop(i)
                else:
                    i += 1
        # o-proj halves: one per slot, right after normalize(qc) completes
        for qc in range(0, N_QC - 1):
            for i in range(8):
                qt = qc * 4 + i // 2
                add_extra(16 * (qc + 1) + LAG + 1 + i,
                          (lambda qt=qt, i=i: oproj_half(qt, i % 2)))

        # main slot loop; proj generators are pumped between waves so
        # their same-region accumulation chains never run back-to-back,
        # and fully drained before the slot their consumer runs in
        for s in range(64):
            qc, kt = divmod(s, 16)
            drain_due(s)
            for fn in extras.get(s, []):
                r = fn()
                if isinstance(r, tuple):
                    pending.append(list(r))
            p01 = pre01[s] if s < 2 else scores_wave(0, qc, kt)
            pump()
            p23 = scores_wave(1, qc, kt)
            pump()
            P[(qc, kt)] = (p01, kt) and (p01, p23)
            if s >= LAG:
                av_stream_elem(s - LAG)
            pump()
            pump()

        # tail: drain everything, then remaining work
        drain_due(10 ** 9)
        for s in range(64, 64 + LAG + 16):
            for fn in extras.get(s, []):
                r = fn()
                if isinstance(r, tuple):
                    for _ in r[1]:
                        pass
        for j in range(64 - LAG, 64):
            av_stream_elem(j)
        normalize_pair(N_QC - 1, 1, avst["den"], avst["au23"])
        for i in range(8):
            oproj_half(12 + i // 2, i % 2, tail=True)
        if dbg is not None:
            nc.sync.dma_start(out=dbg["DKT0"][:], in_=KT[0][:])
            nc.sync.dma_start(out=dbg["DQT0"][:], in_=QT[0][:])
            nc.sync.dma_start(out=dbg["DV"][:], in_=V[:])
            nc.sync.dma_start(out=dbg["DAT0"][:], in_=AT[0][:])


_NC_CACHE = None


def _get_nc():
    global _NC_CACHE
    if _NC_CACHE is None:
        _NC_CACHE = build_kernel()
    return _NC_CACHE


def _pack_w(W, sl):
    """[out,in] torch-layout slice -> SBUF layout [128, 8*256]."""
    WT = np.ascontiguousarray(np.asarray(W, dtype=np.float32)[sl, :].T)
    return np.concatenate([WT[c * 128:(c + 1) * 128, :] for c in range(N_DMC)],
                          axis=1)


def make_in_maps(query, keys, values, Wq, Wk, Wv, Wo):
    import ml_dtypes
    bf16 = ml_dtypes.bfloat16
    query = np.asarray(query, dtype=np.float32)
    keys = np.asarray(keys, dtype=np.float32)
    values = np.asarray(values, dtype=np.float32)
    xTs = {}
    for b in range(BS):
        qTb = query[:, b, :].T          # [DM, SL]
        kTb = keys[:, b, :].T
        vTb = values[:, b, :].T
        q_pieces = np.concatenate(
            [qTb[:, g * 512:(g + 1) * 512] for g in range(N_QC)], axis=0)
        k_pieces = np.concatenate(
            [kTb[:, g * 512:(g + 1) * 512] for g in range(N_KG)], axis=0)
        v_pieces = np.concatenate(
            [vTb[:, g * 128:(g + 1) * 128] for g in range(N_KT)], axis=0)
        xTs[b] = (
            np.ascontiguousarray(q_pieces.astype(np.float16)),
            np.ascontiguousarray(k_pieces.astype(np.float16)),
            np.ascontiguousarray(v_pieces.astype(bf16)),
        )
    wTs = {}
    for g in range(N_CORES // BS):
        sl = slice(g * IC, (g + 1) * IC)
        wTs[g] = (
            np.ascontiguousarray(_pack_w(Wq, sl).astype(np.float16)),
            np.ascontiguousarray(_pack_w(Wk, sl).astype(np.float16)),
            np.ascontiguousarray(_pack_w(Wv, sl).astype(bf16)),
            np.ascontiguousarray(
                np.asarray(Wo, dtype=np.float32)[:, sl].T.astype(np.float16)),
        )
    in_maps = []
    for c in range(N_CORES):
        b, g = c // 4, c % 4
        qTb, kTb, vTb = xTs[b]
        wq, wk, wv, wo = wTs[g]
        in_maps.append({"qT": qTb, "kT": kTb, "vT": vTb,
                        "wqT": wq, "wkT": wk, "wvT": wv, "woT": wo})
    return in_maps


def assemble_output(results):
    out = np.zeros((SL, BS, DM), dtype=np.float32)
    for c in range(N_CORES):
        b = c // 4
        out[:, b, :] += results[c]["Y"].astype(np.float32)
    return out


def kernel(query, keys, values, Wq, Wk, Wv, Wo):
    nc = _get_nc()
    in_maps = make_in_maps(query, keys, values, Wq, Wk, Wv, Wo)
    res = run_bass_kernel_spmd(nc, in_maps, list(range(N_CORES)))
    return assemble_output(res.results)
